# revision 62
# baseline (speedup 1.0000x reference)
"""nn_AttentionOut_63711544869147 — causal multi-head attention + output projection,
distributed over 8 Trainium2 NeuronCores.

Module: out = softmax(causal(Q K^T / sqrt(d))) V @ W_O + b_O, returned with the
(unchanged) residual: reference returns the tuple (residual, out).

Sharding (8 cores = 2 batches x 4 head-groups of 4 heads, SPMD single program):
  each core computes full causal attention for its batch over its 4 heads and
  a partial projection  sum_{h in group} z_h @ W_O[h]  ->  pout [2048, 1024].
  The host sums the 4 head-group partials per batch (the "all-reduce" of the
  row-sharded W_O product), adds b_O, and passes the residual through.

The kernel is emitted as ONE continuous software pipeline over all 80
(strip, head-pair, kv-tile) tiles. Strips are processed LARGEST FIRST
(s=3,2,1,0) so the PE stream is dense from the start (the HAM clock gate
un-throttles early and stays at 8/8) and the pipeline tail is the smallest
pair (4 tiles). A warmup burst of zero matmuls (alternating PE row groups
and PSUM banks, exactly like the real QKs) runs during the initial DMA
phase so the PE clock is warm before the first real QK issues. Input DMAs
are spread across the Sync/Scalar/GpSimd queues in need-time order, with
the ACT queue kept almost empty for the exp stream.

  per tile: scores_T[kv,q] = K^T_tile.T @ Q^T_strip for both heads of the
  pair concurrently (64-deep matmuls in disjoint PE row groups) into TWO
  single-bank [128, 512] fp32 PSUM tiles from a 5-deep ring — the deep
  ring keeps the PE far ahead of the exps, which keeps the matmul stream
  dense and the HAM warm. Each side's exp(scores/8) is its own
  instruction, so the ACT engine (exact exp) and the DVE (Schraudolph
  bf16 bit-trick via one tensor_scalar into int16) each take one side of
  most tiles (~42% of elements on the slower DVE); diagonal tiles get one
  DVE mask-multiply over both heads' straddle blocks.
  PV accumulation (z_ext[65,q] += V_ext.T @ expP, ones column = softmax
  denominator) is emitted with an asymmetric lag: L_HI tiles behind the
  QK/exp stream mid-pair (raised to 10 for a pair's first tiles, whose
  start=True matmul waits on the z-bank ring), catching up to lag L_LO by
  the pair's last tile. The z PSUM ring is 3 deep so consecutive pairs
  overlap their normalize chains.
  normalize (per head, chains independent): denominator row copied
  PSUM->SBUF on ACT, DVE approx-reciprocal, GpSimd partition_broadcast,
  then zn = z * recip on DVE.
  projection (zn_T @ W_O, 256-deep accumulated over both head pairs) is
  event-scheduled ~7 steps after its strip's normalizes (so it never
  head-of-line blocks the next pair's QKs in the PE FIFO), writing two
  single-bank PSUM halves borrowed from the score ring; evacuation copies
  run on ACT and DVE in parallel, and the fp16 row DMAs issue from the
  Sync queue (the last strip's alternate onto ACT so the final drain uses
  two queues).
"""

import numpy as np

import concourse.bass as bass
import concourse.bacc as bacc
import concourse.tile as tile
from concourse import mybir
from concourse.bass_utils import run_bass_kernel_spmd

F32 = mybir.dt.float32
F16 = mybir.dt.float16
I16 = mybir.dt.int16
BF16 = mybir.dt.bfloat16

N_CORES = 8
N_HEADS = 16
H = 4          # heads per core
S = 2048
D = 64
P = 128
D_MODEL = 1024
NSTRIP = 4     # q strips of 512
QW = 512       # strip width
L_HI = 7       # PV lag mid-pair (tiles)
L_LO = 2       # PV lag at pair end
STRIP_ORDER = (3, 2, 1, 0)   # largest strips first
N_WARM = 14    # warmup matmuls (PE clock un-throttle)

# bf16-bit-trick exp: bits16 = rne(score * (0.125*2^7/ln2) + (127*2^7 - 7.25))
SCHR_A = float(0.125 * 128.0 / np.log(2.0))
SCHR_B = float(127 * 128 - 7.25)


def dve_side(s, j, t):
    """Which side of a tile's exp runs on DVE (Schraudolph); None = both on
    ACT.  The DVE is slower per element and carries the normalize work, so
    it takes one side of ~5 out of 6 tiles (~42% of exp elements)."""
    if t % 6 == 5:
        return None
    return (t + j) % 2


def build_program():
    MMDT = BF16
    nc = bacc.Bacc(target_bir_lowering=False)

    qT = nc.dram_tensor("qT", [H, D, S], MMDT, kind="ExternalInput")
    kT = nc.dram_tensor("kT", [H, D, S], MMDT, kind="ExternalInput")
    vx = nc.dram_tensor("vx", [H, P, 16, D + 1], MMDT, kind="ExternalInput")
    wo = nc.dram_tensor("wo", [2 * P, D_MODEL], MMDT, kind="ExternalInput")
    tri2 = nc.dram_tensor("tri2", [P, 2 * P], MMDT, kind="ExternalInput")
    pout = nc.dram_tensor("pout", [S, D_MODEL], F16, kind="ExternalOutput")

    with tile.TileContext(nc) as tc:
        with (
            tc.tile_pool(name="persist", bufs=1) as persist,
            tc.tile_pool(name="expp", bufs=16) as expp,
            tc.tile_pool(name="rcpp", bufs=4) as rcpp,
            tc.tile_pool(name="rbp", bufs=4) as rbp,
            tc.tile_pool(name="outp", bufs=6) as outp,
            tc.tile_pool(name="znp", bufs=6) as znp,
            tc.tile_pool(name="scps", bufs=5, space="PSUM") as scps,
            tc.tile_pool(name="zps", bufs=3, space="PSUM") as zps,
        ):
            # ---- PE warmup: zero matmuls during the DMA phase so the HAM
            # clock gate is at 8/8 before the first real QK issues. The MMs
            # alternate PE row groups (partitions 0 / 32) so each LDWEIGHTS
            # overlaps the in-flight matmul and the burst is gap-free ----
            warm_sb = persist.tile([P, 640], MMDT, tag="warm", name="warm_sb")
            nc.vector.memzero(warm_sb[:])
            warm_ps = [
                scps.tile([P, QW], F32, tag="sc", name=f"warm_ps{i}")
                for i in range(2)
            ]
            for i in range(N_WARM):
                # same row-group + output alternation as the real QKs: the
                # concurrent row-group matmuls must write disjoint PSUM
                b = D * (i % 2)
                nc.tensor.matmul(
                    warm_ps[i % 2][:],
                    warm_sb[b : b + D, 0:P],
                    warm_sb[b : b + D, P : P + QW],
                    start=True, stop=True,
                )

            # ---- persistent loads (everything stays SBUF-resident) ----
            # spread across the three DMA-capable queues; ordered so strip 3 /
            # head-pair 0 can start as early as possible
            qT_sb = [None, None]
            kT_sb = [None, None]
            wo_sb = [None, None]
            vext_sb = [None] * H

            for j in range(2):
                kT_sb[j] = persist.tile([P, S], MMDT, tag=f"kT{j}", name=f"kT{j}")
                qT_sb[j] = persist.tile([P, S], MMDT, tag=f"qT{j}", name=f"qT{j}")
            tri_sb = persist.tile([P, 2, P], MMDT, tag="tri", name="tri_sb")
            for h in range(H):
                vext_sb[h] = persist.tile([P, 16, D + 1], MMDT, tag=f"vext{h}", name=f"vext{h}")

            # strip 3 (processed first) needs qT cols 1536:2048 immediately,
            # kv tiles progressively from 0, and vext[0]/vext[1] by ~step 7.
            # Every queue issues in need-time order; the vext loads are
            # spread across all three queues (serialized on one queue they
            # arrive after the PV matmuls want them and stall the PE FIFO).
            CH = 512

            def kT_chunk(eng, j, c):
                eng.dma_start(
                    kT_sb[j][:, c * CH : (c + 1) * CH],
                    kT[2 * j : 2 * j + 2, :, c * CH : (c + 1) * CH]
                    .rearrange("h d s -> (h d) s"))

            def qT_chunk(eng, j, c):
                eng.dma_start(
                    qT_sb[j][:, c * CH : (c + 1) * CH],
                    qT[2 * j : 2 * j + 2, :, c * CH : (c + 1) * CH]
                    .rearrange("h d s -> (h d) s"))

            # the ACT queue must stay clear for the exp stream: it gets ONLY
            # the two immediately-needed qT chunks; sync and gpsimd carry the
            # rest in need-time order
            qT_chunk(nc.scalar, 0, 3)
            qT_chunk(nc.scalar, 1, 3)

            kT_chunk(nc.sync, 0, 0)
            kT_chunk(nc.sync, 0, 1)
            nc.sync.dma_start(vext_sb[1][:], vx[1])
            kT_chunk(nc.sync, 0, 2)
            kT_chunk(nc.sync, 0, 3)
            kT_chunk(nc.sync, 1, 2)
            kT_chunk(nc.sync, 1, 3)
            for j in range(2):
                wo_sb[j] = persist.tile([P, D_MODEL], MMDT, tag=f"wo{j}", name=f"wo{j}")
                nc.sync.dma_start(wo_sb[j][:], wo[P * j : P * (j + 1), :])
            qT_chunk(nc.sync, 0, 2)
            qT_chunk(nc.sync, 1, 2)

            nc.gpsimd.dma_start(tri_sb[:], tri2[:].rearrange("p (a b) -> p a b", a=2))
            nc.gpsimd.dma_start(vext_sb[0][:], vx[0])
            kT_chunk(nc.gpsimd, 1, 0)
            kT_chunk(nc.gpsimd, 1, 1)
            nc.gpsimd.dma_start(vext_sb[2][:], vx[2])
            nc.gpsimd.dma_start(vext_sb[3][:], vx[3])
            qT_chunk(nc.gpsimd, 0, 1)
            qT_chunk(nc.gpsimd, 1, 1)
            qT_chunk(nc.gpsimd, 0, 0)
            qT_chunk(nc.gpsimd, 1, 0)

            # ---- global pipeline schedule ----
            # strips 3..1: each strip's two pairs run back to back.  The
            # LAST strip's two pairs are interleaved step-wise and the j1
            # pair's z accumulators borrow score-ring banks, so the two
            # normalize chains overlap instead of serializing through the
            # z ring — this directly shortens the kernel tail.
            tiles = []        # (pair_idx, t)
            pair_info = []    # per pair: s, j, nt, z source
            for s in STRIP_ORDER:
                nt = 4 * s + 4
                base = len(pair_info)
                for j in range(2):
                    pair_info.append(
                        {"s": s, "j": j, "nt": nt,
                         "zsc": (s == 0 and j == 1),
                         "z": None, "ex": {}, "zn": None}
                    )
                if s == 0:
                    for t in range(nt):
                        tiles.append((base, t))
                        tiles.append((base + 1, t))
                else:
                    for j in range(2):
                        for t in range(nt):
                            tiles.append((base + j, t))
            n_steps = len(tiles)
            gend = {}         # pair -> global step of its last tile
            for g, (p, t) in enumerate(tiles):
                gend[p] = g
            zn_strip = {}     # strip -> [zn_j0, zn_j1]
            actions = {}      # step -> list of closures (after the step's QK)
            pre_actions = {}  # step -> list of closures (before the step's QK)

            def at(step, fn):
                actions.setdefault(step, []).append(fn)

            def at_pre(step, fn):
                pre_actions.setdefault(step, []).append(fn)

            def emit_qk_exp(p, t):
                info = pair_info[p]
                s, j, nt = info["s"], info["j"], info["nt"]
                q0 = s * QW
                li = max(0, (t - 4 * s)) * P
                if t == 0:
                    if info["zsc"]:
                        # borrowed single-bank score-ring tiles (rows 0..64
                        # used); lets this pair accumulate concurrently with
                        # the other interleaved pair's z-ring tiles
                        info["z"] = [
                            scps.tile([P, QW], F32, tag="sc", name=f"zb{x}")
                            for x in ("A", "B")
                        ]
                    else:
                        info["z"] = [
                            zps.tile([D + 1, QW], F32, tag="z", name=f"z{x}")
                            for x in ("A", "B")
                        ]
                if info["zn"] is None:
                    zn = [
                        znp.tile([P, QW], MMDT, tag=f"zn{jj}", name=f"zn{jj}")
                        for jj in range(2)
                    ] if j == 0 else zn_strip[s]
                    zn_strip[s] = zn
                    info["zn"] = zn
                # per-side 1-bank score tiles: the two heads' 64-deep QK
                # matmuls target disjoint PE row groups (rows 0-63 / 64-127)
                # and run concurrently into disjoint PSUM banks; each side's
                # exp is its own instruction, so the two engines share every
                # tile's exp work and the per-tile exp latency halves
                ex = expp.tile([P, 2, QW], MMDT, tag="ex", name="ex")
                for side in (0, 1):
                    off = side * D
                    sc = scps.tile([P, QW], F32, tag="sc", name="sc")
                    nc.tensor.matmul(
                        sc[:, li:QW],
                        kT_sb[j][off : off + D, t * P : (t + 1) * P],
                        qT_sb[j][off : off + D, q0 + li : q0 + QW],
                        start=True,
                        stop=True,
                    )
                    if side == dve_side(s, j, t):
                        nc.vector.tensor_scalar(
                            ex[:, side, li:QW].bitcast(I16), sc[:, li:QW],
                            SCHR_A, SCHR_B,
                            mybir.AluOpType.mult, mybir.AluOpType.add,
                        )
                    else:
                        nc.scalar.activation(
                            ex[:, side, li:QW], sc[:, li:QW],
                            mybir.ActivationFunctionType.Exp, scale=0.125,
                        )
                info["ex"][t] = ex

            def emit_mask(p, t):
                """one mask multiply over both heads' straddle blocks;
                scheduled two steps after the exp so the Vector queue never
                blocks waiting for an ACT-engine exp"""
                info = pair_info[p]
                li = (t - 4 * info["s"]) * P
                m = info["ex"][t][:, :, li : li + P]
                nc.vector.tensor_mul(m, m, tri_sb[:])

            def emit_pv(p, t):
                info = pair_info[p]
                s, nt = info["s"], info["nt"]
                li = max(0, (t - 4 * s)) * P
                ex = info["ex"].pop(t)
                for side, h in ((0, 2 * info["j"]), (1, 2 * info["j"] + 1)):
                    nc.tensor.matmul(
                        info["z"][side][0 : D + 1, li:QW],
                        vext_sb[h][:, t, :],
                        ex[:, side, li:QW],
                        start=(t == 0),
                        stop=(t == nt - 1),
                    )

            def emit_norm1(p):
                """denominator -> reciprocal -> partition broadcast.
                The PSUM->SBUF staging copies run on ACT (cheap there); the
                approx reciprocal is DVE-only; the broadcast is a GpSimd
                custom op (the DVE can't take 0-stride partition APs).
                The two sides' chains stay fully independent — joining them
                lengthens the pair-boundary critical path."""
                info = pair_info[p]
                info["rb"] = []
                for side in (0, 1):
                    dcp = rcpp.tile([1, QW], F32, tag="dcp", name="dcp")
                    nc.scalar.copy(dcp[:], info["z"][side][D : D + 1, :])
                    rcp = rcpp.tile([1, QW], F32, tag="rcp", name="rcp")
                    # (custom-DVE op requires an SBUF input; PSUM reads garbage)
                    nc.vector.reciprocal_approx_fast(rcp[:], dcp[:])
                    rb_sb = rbp.tile([D, QW], F32, tag="rb_sb", name="rb_sb")
                    nc.gpsimd.partition_broadcast(rb_sb[:], rcp[:])
                    info["rb"].append(rb_sb)

            def emit_norm2(p):
                """zn = z * (1/denom); frees the pair's z banks"""
                info = pair_info[p]
                j = info["j"]
                for side in (0, 1):
                    off = side * D
                    nc.vector.tensor_mul(
                        info["zn"][j][off : off + 64, :],
                        info["z"][side][0:D, :],
                        info["rb"][side][:],
                    )

            proj_tiles = {}

            def emit_proj_mms(s, qb):
                zn_sb = zn_strip[s]
                # projection borrows two 1-bank tiles from the score ring
                # (one per 512-wide half of d_model)
                ops = [
                    scps.tile([P, 512], F32, tag="sc", name="wo_ps")
                    for _ in range(2)
                ]
                proj_tiles[(s, qb)] = ops
                for j2 in range(2):
                    for mt in range(2):
                        nc.tensor.matmul(
                            ops[mt][:],
                            zn_sb[j2][:, qb * P : (qb + 1) * P],
                            wo_sb[j2][:, mt * 512 : (mt + 1) * 512],
                            start=(j2 == 0),
                            stop=(j2 == 1),
                        )

            def emit_proj_copies(s, qb):
                ops = proj_tiles.pop((s, qb))
                ot = outp.tile([P, 2, 512], F16, tag="ot", name="ot")
                # the two halves' evacuation copies run on BOTH queues in
                # parallel, then one merged row DMA on the Sync queue
                nc.scalar.copy(ot[:, 0, :], ops[0][:])
                nc.vector.tensor_copy(ot[:, 1, :], ops[1][:])
                # the final strip's stores drain on two queues in parallel so
                # the end-of-kernel DMA flush is shorter (mid-kernel stores
                # stay off the ACT queue, which feeds the exps)
                eng = nc.scalar if (s == 0 and qb % 2 == 1) else nc.sync
                eng.dma_start(
                    pout[(4 * s + qb) * P : (4 * s + qb + 1) * P, :],
                    ot[:].rearrange("p a b -> p (a b)"),
                )

            # schedule PVs with asymmetric lag + boundary events.  The lag is
            # raised further for a pair's first tiles: the pair's first PV
            # (start=True) waits on the PREVIOUS pair's zn to free the z-bank
            # ring, so extra QK runway must sit ahead of it in the PE FIFO.
            for g, (p, t) in enumerate(tiles):
                info = pair_info[p]
                nt = info["nt"]
                if t >= 4 * info["s"]:
                    at(g + 2, (lambda p=p, t=t: emit_mask(p, t)))
                if info["s"] == 0:
                    # tail pairs: short lag so the PVs fill the otherwise
                    # idling PE, and a tight cap so normalize starts ASAP
                    lag, cap = 4, gend[p] + 1
                else:
                    lag, cap = max(L_HI, 10 - t), gend[p] + L_LO
                f = max(g + 2, min(g + lag, cap))
                at(f, (lambda p=p, t=t: emit_pv(p, t)))
                if t == nt - 1:
                    at(f, (lambda p=p: emit_norm1(p)))
                    at(f + 3, (lambda p=p: emit_norm2(p)))
                    if info["j"] == 1:
                        # the proj matmuls also wait on both pairs' zn; defer
                        # them past the normalize-chain latency so they don't
                        # head-of-line block the next pair's QKs (except for
                        # the final strip, where nothing follows)
                        d = 4 if info["s"] == 0 else 7
                        for qb in range(4):
                            # 2-step spacing: block qb+1's matmuls reuse
                            # qb's banks, so qb's copies must be emitted
                            # first
                            at(f + d + 2 * qb,
                               (lambda s=info["s"], qb=qb:
                                emit_proj_mms(s, qb)))
                            at(f + d + 2 + 2 * qb,
                               (lambda s=info["s"], qb=qb:
                                emit_proj_copies(s, qb)))

            # ---- run the pipeline ----
            for g in range(n_steps):
                p, t = tiles[g]
                for fn in pre_actions.pop(g, ()):
                    fn()
                emit_qk_exp(p, t)
                for fn in actions.pop(g, ()):
                    fn()
            for g in sorted(set(actions) | set(pre_actions)):
                for fn in pre_actions.pop(g, ()):
                    fn()
                for fn in actions.pop(g, ()):
                    fn()

    nc.finalize()
    return nc


_PROGRAM = None
LAST_RESULTS = None


def _get_program():
    global _PROGRAM
    if _PROGRAM is None:
        _PROGRAM = build_program()
    return _PROGRAM


def make_in_maps(q, k, v, W_O, n_cores=N_CORES):
    """Shard full inputs into per-core maps (core = batch*4 + head_group)."""
    import ml_dtypes
    mmdt = ml_dtypes.bfloat16
    q = np.ascontiguousarray(np.asarray(q, dtype=np.float32))
    k = np.ascontiguousarray(np.asarray(k, dtype=np.float32))
    v = np.ascontiguousarray(np.asarray(v, dtype=np.float32))
    W_O = np.ascontiguousarray(np.asarray(W_O, dtype=np.float32))
    B = q.shape[0]
    qT = np.ascontiguousarray(q.reshape(B, S, N_HEADS, D).transpose(0, 2, 3, 1))
    kT = np.ascontiguousarray(k.reshape(B, S, N_HEADS, D).transpose(0, 2, 3, 1))
    # v extended with a ones column (softmax denominator row) and pre-arranged
    # to the on-chip [partition, kv_tile, d+1] layout so the DMA is contiguous
    vh = v.reshape(B, S, N_HEADS, D).transpose(0, 2, 1, 3)  # [B, H, S, D]
    vext = np.concatenate(
        [vh, np.ones((B, N_HEADS, S, 1), dtype=np.float32)], axis=3
    ).reshape(B, N_HEADS, 16, P, D + 1).transpose(0, 1, 3, 2, 4)  # [B, Hh, P, 16, D+1]
    # mask[kv, q] = 1 iff kv <= q  (scores live transposed: partition=kv, free=q)
    tri = np.triu(np.ones((P, P), dtype=np.float32))
    tri2 = np.ascontiguousarray(np.concatenate([tri, tri], axis=1))
    in_maps = []
    for core in range(n_cores):
        b, g = core // 4, core % 4
        hs = slice(H * g, H * (g + 1))
        in_maps.append(
            {
                "qT": np.ascontiguousarray(qT[b, hs]).astype(mmdt),
                "kT": np.ascontiguousarray(kT[b, hs]).astype(mmdt),
                "vx": np.ascontiguousarray(vext[b, hs]).astype(mmdt),
                "wo": np.ascontiguousarray(W_O[hs].reshape(2 * P, D_MODEL)).astype(mmdt),
                "tri2": tri2.astype(mmdt),
            }
        )
    return in_maps


def kernel(residual, q, k, v, W_O, b_O, _trace=False, _trace_kwargs=None):
    global LAST_RESULTS
    residual = np.asarray(residual, dtype=np.float32)
    B = residual.shape[0]
    in_maps = make_in_maps(q, k, v, W_O)
    nc = _get_program()
    res = run_bass_kernel_spmd(
        nc, in_maps, list(range(N_CORES)), trace=_trace, **(_trace_kwargs or {})
    )
    LAST_RESULTS = res
    out = np.zeros((B, S, D_MODEL), dtype=np.float32)
    for core in range(N_CORES):
        out[core // 4] += res.results[core]["pout"].astype(np.float32)
    out += np.asarray(b_O, dtype=np.float32)
    return (residual, out.astype(np.float32))


# revision 64
# speedup vs baseline: 1.0589x; 1.0589x over previous
"""nn_AttentionOut_63711544869147 — causal multi-head attention + output projection,
distributed over 8 Trainium2 NeuronCores.

Module: out = softmax(causal(Q K^T / sqrt(d))) V @ W_O + b_O, returned with the
(unchanged) residual: reference returns the tuple (residual, out).

Sharding (8 cores = 2 batches x 4 head-groups of 4 heads, SPMD single program):
  each core computes full causal attention for its batch over its 4 heads and
  a partial projection  sum_{h in group} z_h @ W_O[h]  ->  pout [2048, 1024].
  The host sums the 4 head-group partials per batch (the "all-reduce" of the
  row-sharded W_O product), adds b_O, and passes the residual through.

The kernel is emitted as ONE continuous software pipeline over all 80
(strip, head-pair, kv-tile) tiles. Strips are processed LARGEST FIRST
(s=3,2,1,0) so the PE stream is dense from the start (the HAM clock gate
un-throttles early and stays at 8/8) and the pipeline tail is the smallest
pair (4 tiles). A warmup burst of zero matmuls (alternating PE row groups
and PSUM banks, exactly like the real QKs) runs during the initial DMA
phase so the PE clock is warm before the first real QK issues. Input DMAs
are spread across the Sync/Scalar/GpSimd queues in need-time order, with
the ACT queue kept almost empty for the exp stream.

  per tile: scores_T[kv,q] = K^T_tile.T @ Q^T_strip for both heads of the
  pair concurrently (64-deep matmuls in disjoint PE row groups) into TWO
  single-bank [128, 512] fp32 PSUM tiles from a 5-deep ring — the deep
  ring keeps the PE far ahead of the exps, which keeps the matmul stream
  dense and the HAM warm. Each side's exp(scores/8) is its own
  instruction, so the ACT engine (exact exp) and the DVE (Schraudolph
  bf16 bit-trick via one tensor_scalar into int16) each take one side of
  most tiles (~42% of elements on the slower DVE); diagonal tiles get one
  DVE mask-multiply over both heads' straddle blocks.
  PV accumulation (z_ext[65,q] += V_ext.T @ expP, ones column = softmax
  denominator) is emitted with an asymmetric lag: L_HI tiles behind the
  QK/exp stream mid-pair (raised to 10 for a pair's first tiles, whose
  start=True matmul waits on the z-bank ring), catching up to lag L_LO by
  the pair's last tile. The z PSUM ring is 3 deep so consecutive pairs
  overlap their normalize chains.
  normalize (per head, chains independent): denominator row copied
  PSUM->SBUF on ACT, DVE approx-reciprocal, GpSimd partition_broadcast,
  then zn = z * recip on DVE.
  projection (zn_T @ W_O, 256-deep accumulated over both head pairs) is
  event-scheduled ~7 steps after its strip's normalizes (so it never
  head-of-line blocks the next pair's QKs in the PE FIFO), writing two
  single-bank PSUM halves borrowed from the score ring; evacuation copies
  run on ACT and DVE in parallel, and the fp16 row DMAs issue from the
  Sync queue (the last strip's alternate onto ACT so the final drain uses
  two queues).
"""

import numpy as np

import concourse.bass as bass
import concourse.bacc as bacc
import concourse.tile as tile
from concourse import mybir
from concourse.bass_utils import run_bass_kernel_spmd

F32 = mybir.dt.float32
F16 = mybir.dt.float16
I16 = mybir.dt.int16
BF16 = mybir.dt.bfloat16

N_CORES = 8
N_HEADS = 16
H = 4          # heads per core
S = 2048
D = 64
P = 128
D_MODEL = 1024
NSTRIP = 4     # q strips of 512
QW = 512       # strip width
L_HI = 7       # PV lag mid-pair (tiles)
L_LO = 2       # PV lag at pair end
STRIP_ORDER = (3, 2, 1, 0)   # largest strips first
N_WARM = 14    # warmup matmuls (PE clock un-throttle)

# bf16-bit-trick exp: bits16 = rne(score * (0.125*2^7/ln2) + (127*2^7 - 7.25))
SCHR_A = float(0.125 * 128.0 / np.log(2.0))
SCHR_B = float(127 * 128 - 7.25)


def dve_side(s, j, t):
    """Which side of a tile's exp runs on DVE (Schraudolph); None = both on
    ACT.  The DVE is slower per element and carries the normalize work, so
    it takes one side of ~5 out of 6 tiles (~42% of exp elements)."""
    if t % 6 == 5:
        return None
    return (t + j) % 2


def build_program():
    MMDT = BF16
    nc = bacc.Bacc(target_bir_lowering=False)

    qT = nc.dram_tensor("qT", [H, D, S], MMDT, kind="ExternalInput")
    kT = nc.dram_tensor("kT", [H, D, S], MMDT, kind="ExternalInput")
    vx = nc.dram_tensor("vx", [H, P, 16, D + 1], MMDT, kind="ExternalInput")
    wo = nc.dram_tensor("wo", [2 * P, D_MODEL], MMDT, kind="ExternalInput")
    tri2 = nc.dram_tensor("tri2", [P, 2 * P], MMDT, kind="ExternalInput")
    pout = nc.dram_tensor("pout", [S, D_MODEL], F16, kind="ExternalOutput")

    with tile.TileContext(nc) as tc:
        with (
            tc.tile_pool(name="persist", bufs=1) as persist,
            tc.tile_pool(name="expp", bufs=16) as expp,
            tc.tile_pool(name="rcpp", bufs=4) as rcpp,
            tc.tile_pool(name="rbp", bufs=4) as rbp,
            tc.tile_pool(name="outp", bufs=6) as outp,
            tc.tile_pool(name="znp", bufs=6) as znp,
            tc.tile_pool(name="scps", bufs=5, space="PSUM") as scps,
            tc.tile_pool(name="zps", bufs=3, space="PSUM") as zps,
        ):
            # ---- PE warmup: zero matmuls during the DMA phase so the HAM
            # clock gate is at 8/8 before the first real QK issues. The MMs
            # alternate PE row groups (partitions 0 / 32) so each LDWEIGHTS
            # overlaps the in-flight matmul and the burst is gap-free ----
            warm_sb = persist.tile([P, 640], MMDT, tag="warm", name="warm_sb")
            nc.vector.memzero(warm_sb[:])
            warm_ps = [
                scps.tile([P, QW], F32, tag="sc", name=f"warm_ps{i}")
                for i in range(2)
            ]
            for i in range(N_WARM):
                # same row-group + output alternation as the real QKs: the
                # concurrent row-group matmuls must write disjoint PSUM
                b = D * (i % 2)
                nc.tensor.matmul(
                    warm_ps[i % 2][:],
                    warm_sb[b : b + D, 0:P],
                    warm_sb[b : b + D, P : P + QW],
                    start=True, stop=True,
                )

            # ---- persistent loads (everything stays SBUF-resident) ----
            # spread across the three DMA-capable queues; ordered so strip 3 /
            # head-pair 0 can start as early as possible
            qT_sb = [None, None]
            kT_sb = [None, None]
            wo_sb = [None, None]
            vext_sb = [None] * H

            for j in range(2):
                kT_sb[j] = persist.tile([P, S], MMDT, tag=f"kT{j}", name=f"kT{j}")
                qT_sb[j] = persist.tile([P, S], MMDT, tag=f"qT{j}", name=f"qT{j}")
            tri_sb = persist.tile([P, 2, P], MMDT, tag="tri", name="tri_sb")
            for h in range(H):
                vext_sb[h] = persist.tile([P, 16, D + 1], MMDT, tag=f"vext{h}", name=f"vext{h}")

            # strip 3 (processed first) needs qT cols 1536:2048 immediately,
            # kv tiles progressively from 0, and vext[0]/vext[1] by ~step 7.
            # Every queue issues in need-time order; the vext loads are
            # spread across all three queues (serialized on one queue they
            # arrive after the PV matmuls want them and stall the PE FIFO).
            CH = 512

            def kT_chunk(eng, j, c):
                eng.dma_start(
                    kT_sb[j][:, c * CH : (c + 1) * CH],
                    kT[2 * j : 2 * j + 2, :, c * CH : (c + 1) * CH]
                    .rearrange("h d s -> (h d) s"))

            def qT_chunk(eng, j, c):
                eng.dma_start(
                    qT_sb[j][:, c * CH : (c + 1) * CH],
                    qT[2 * j : 2 * j + 2, :, c * CH : (c + 1) * CH]
                    .rearrange("h d s -> (h d) s"))

            # the ACT queue must stay clear for the exp stream: it gets ONLY
            # the two immediately-needed qT chunks; sync and gpsimd carry the
            # rest in need-time order
            qT_chunk(nc.scalar, 0, 3)
            qT_chunk(nc.scalar, 1, 3)

            kT_chunk(nc.sync, 0, 0)
            kT_chunk(nc.sync, 0, 1)
            nc.sync.dma_start(vext_sb[1][:], vx[1])
            kT_chunk(nc.sync, 0, 2)
            kT_chunk(nc.sync, 0, 3)
            kT_chunk(nc.sync, 1, 2)
            kT_chunk(nc.sync, 1, 3)
            for j in range(2):
                wo_sb[j] = persist.tile([P, D_MODEL], MMDT, tag=f"wo{j}", name=f"wo{j}")
                nc.sync.dma_start(wo_sb[j][:], wo[P * j : P * (j + 1), :])
            qT_chunk(nc.sync, 0, 2)
            qT_chunk(nc.sync, 1, 2)

            nc.gpsimd.dma_start(tri_sb[:], tri2[:].rearrange("p (a b) -> p a b", a=2))
            nc.gpsimd.dma_start(vext_sb[0][:], vx[0])
            kT_chunk(nc.gpsimd, 1, 0)
            kT_chunk(nc.gpsimd, 1, 1)
            nc.gpsimd.dma_start(vext_sb[2][:], vx[2])
            nc.gpsimd.dma_start(vext_sb[3][:], vx[3])
            qT_chunk(nc.gpsimd, 0, 1)
            qT_chunk(nc.gpsimd, 1, 1)
            qT_chunk(nc.gpsimd, 0, 0)
            qT_chunk(nc.gpsimd, 1, 0)

            # ---- global pipeline schedule ----
            # strips 3..1: each strip's two pairs run back to back.  The
            # LAST strip's two pairs are interleaved step-wise and the j1
            # pair's z accumulators borrow score-ring banks, so the two
            # normalize chains overlap instead of serializing through the
            # z ring — this directly shortens the kernel tail.
            tiles = []        # (pair_idx, t)
            pair_info = []    # per pair: s, j, nt
            for s in STRIP_ORDER:
                nt = 4 * s + 4
                base = len(pair_info)
                for j in range(2):
                    pair_info.append(
                        {"s": s, "j": j, "nt": nt, "zsc": False,
                         "z": None, "ex": {}, "zn": None}
                    )
                for j in range(2):
                    for t in range(nt):
                        tiles.append((base + j, t))
            n_steps = len(tiles)
            gend = {}         # pair -> global step of its last tile
            for g, (p, t) in enumerate(tiles):
                gend[p] = g
            zn_strip = {}     # strip -> [zn_j0, zn_j1]
            actions = {}      # step -> list of closures (after the step's QK)
            pre_actions = {}  # step -> list of closures (before the step's QK)

            def at(step, fn):
                actions.setdefault(step, []).append(fn)

            def at_pre(step, fn):
                pre_actions.setdefault(step, []).append(fn)

            def emit_qk_exp(p, t):
                info = pair_info[p]
                s, j, nt = info["s"], info["j"], info["nt"]
                q0 = s * QW
                li = max(0, (t - 4 * s)) * P
                if t == 0:
                    if info["zsc"]:
                        # borrowed single-bank score-ring tiles (rows 0..64
                        # used); lets this pair accumulate concurrently with
                        # the other interleaved pair's z-ring tiles
                        info["z"] = [
                            scps.tile([P, QW], F32, tag="sc", name=f"zb{x}")
                            for x in ("A", "B")
                        ]
                    else:
                        info["z"] = [
                            zps.tile([D + 1, QW], F32, tag="z", name=f"z{x}")
                            for x in ("A", "B")
                        ]
                if info["zn"] is None:
                    zn = [
                        znp.tile([P, QW], MMDT, tag=f"zn{jj}", name=f"zn{jj}")
                        for jj in range(2)
                    ] if j == 0 else zn_strip[s]
                    zn_strip[s] = zn
                    info["zn"] = zn
                # per-side 1-bank score tiles: the two heads' 64-deep QK
                # matmuls target disjoint PE row groups (rows 0-63 / 64-127)
                # and run concurrently into disjoint PSUM banks; each side's
                # exp is its own instruction, so the two engines share every
                # tile's exp work and the per-tile exp latency halves
                ex = expp.tile([P, 2, QW], MMDT, tag="ex", name="ex")
                for side in (0, 1):
                    off = side * D
                    sc = scps.tile([P, QW], F32, tag="sc", name="sc")
                    nc.tensor.matmul(
                        sc[:, li:QW],
                        kT_sb[j][off : off + D, t * P : (t + 1) * P],
                        qT_sb[j][off : off + D, q0 + li : q0 + QW],
                        start=True,
                        stop=True,
                    )
                    if side == dve_side(s, j, t):
                        nc.vector.tensor_scalar(
                            ex[:, side, li:QW].bitcast(I16), sc[:, li:QW],
                            SCHR_A, SCHR_B,
                            mybir.AluOpType.mult, mybir.AluOpType.add,
                        )
                    else:
                        nc.scalar.activation(
                            ex[:, side, li:QW], sc[:, li:QW],
                            mybir.ActivationFunctionType.Exp, scale=0.125,
                        )
                info["ex"][t] = ex

            def emit_mask(p, t):
                """one mask multiply over both heads' straddle blocks;
                scheduled two steps after the exp so the Vector queue never
                blocks waiting for an ACT-engine exp"""
                info = pair_info[p]
                li = (t - 4 * info["s"]) * P
                m = info["ex"][t][:, :, li : li + P]
                nc.vector.tensor_mul(m, m, tri_sb[:])

            def emit_pv(p, t):
                info = pair_info[p]
                s, nt = info["s"], info["nt"]
                li = max(0, (t - 4 * s)) * P
                ex = info["ex"].pop(t)
                for side, h in ((0, 2 * info["j"]), (1, 2 * info["j"] + 1)):
                    nc.tensor.matmul(
                        info["z"][side][0 : D + 1, li:QW],
                        vext_sb[h][:, t, :],
                        ex[:, side, li:QW],
                        start=(t == 0),
                        stop=(t == nt - 1),
                    )

            def emit_norm1(p):
                """denominator -> reciprocal -> partition broadcast.
                The PSUM->SBUF staging copies run on ACT (cheap there); the
                approx reciprocal is DVE-only; the broadcast is a GpSimd
                custom op (the DVE can't take 0-stride partition APs).
                The two sides' chains stay fully independent — joining them
                lengthens the pair-boundary critical path."""
                info = pair_info[p]
                info["rb"] = []
                for side in (0, 1):
                    dcp = rcpp.tile([1, QW], F32, tag="dcp", name="dcp")
                    nc.scalar.copy(dcp[:], info["z"][side][D : D + 1, :])
                    rcp = rcpp.tile([1, QW], F32, tag="rcp", name="rcp")
                    # (custom-DVE op requires an SBUF input; PSUM reads garbage)
                    nc.vector.reciprocal_approx_fast(rcp[:], dcp[:])
                    rb_sb = rbp.tile([D, QW], F32, tag="rb_sb", name="rb_sb")
                    nc.gpsimd.partition_broadcast(rb_sb[:], rcp[:])
                    info["rb"].append(rb_sb)

            def emit_norm2(p):
                """zn = z * (1/denom); frees the pair's z banks"""
                info = pair_info[p]
                j = info["j"]
                for side in (0, 1):
                    off = side * D
                    nc.vector.tensor_mul(
                        info["zn"][j][off : off + 64, :],
                        info["z"][side][0:D, :],
                        info["rb"][side][:],
                    )

            proj_tiles = {}

            def emit_proj_mms(s, qb):
                zn_sb = zn_strip[s]
                # projection borrows two 1-bank tiles from the score ring
                # (one per 512-wide half of d_model)
                ops = [
                    scps.tile([P, 512], F32, tag="sc", name="wo_ps")
                    for _ in range(2)
                ]
                proj_tiles[(s, qb)] = ops
                for j2 in range(2):
                    for mt in range(2):
                        nc.tensor.matmul(
                            ops[mt][:],
                            zn_sb[j2][:, qb * P : (qb + 1) * P],
                            wo_sb[j2][:, mt * 512 : (mt + 1) * 512],
                            start=(j2 == 0),
                            stop=(j2 == 1),
                        )

            def emit_proj_copies(s, qb):
                ops = proj_tiles.pop((s, qb))
                ot = outp.tile([P, 2, 512], F16, tag="ot", name="ot")
                # the two halves' evacuation copies run on BOTH queues in
                # parallel, then one merged row DMA on the Sync queue
                nc.scalar.copy(ot[:, 0, :], ops[0][:])
                nc.vector.tensor_copy(ot[:, 1, :], ops[1][:])
                # the final strip's stores drain on two queues in parallel so
                # the end-of-kernel DMA flush is shorter (mid-kernel stores
                # stay off the ACT queue, which feeds the exps)
                eng = nc.scalar if (s == 0 and qb % 2 == 1) else nc.sync
                eng.dma_start(
                    pout[(4 * s + qb) * P : (4 * s + qb + 1) * P, :],
                    ot[:].rearrange("p a b -> p (a b)"),
                )

            # schedule PVs with asymmetric lag + boundary events.  The lag is
            # raised further for a pair's first tiles: the pair's first PV
            # (start=True) waits on the PREVIOUS pair's zn to free the z-bank
            # ring, so extra QK runway must sit ahead of it in the PE FIFO.
            for g, (p, t) in enumerate(tiles):
                info = pair_info[p]
                nt = info["nt"]
                if t >= 4 * info["s"]:
                    at(g + 2, (lambda p=p, t=t: emit_mask(p, t)))
                lag, cap = max(L_HI, 10 - t), gend[p] + L_LO
                f = max(g + 2, min(g + lag, cap))
                at(f, (lambda p=p, t=t: emit_pv(p, t)))
                if t == nt - 1:
                    at(f, (lambda p=p: emit_norm1(p)))
                    at(f + 3, (lambda p=p: emit_norm2(p)))
                    if info["j"] == 1:
                        # the proj matmuls also wait on both pairs' zn; defer
                        # them past the normalize-chain latency so they don't
                        # head-of-line block the next pair's QKs (except for
                        # the final strip, where nothing follows)
                        d = 4 if info["s"] == 0 else 7
                        for qb in range(4):
                            # 2-step spacing: block qb+1's matmuls reuse
                            # qb's banks, so qb's copies must be emitted
                            # first
                            at(f + d + 2 * qb,
                               (lambda s=info["s"], qb=qb:
                                emit_proj_mms(s, qb)))
                            at(f + d + 2 + 2 * qb,
                               (lambda s=info["s"], qb=qb:
                                emit_proj_copies(s, qb)))

            # ---- run the pipeline ----
            for g in range(n_steps):
                p, t = tiles[g]
                for fn in pre_actions.pop(g, ()):
                    fn()
                emit_qk_exp(p, t)
                for fn in actions.pop(g, ()):
                    fn()
            for g in sorted(set(actions) | set(pre_actions)):
                for fn in pre_actions.pop(g, ()):
                    fn()
                for fn in actions.pop(g, ()):
                    fn()

    nc.finalize()
    return nc


_PROGRAM = None
LAST_RESULTS = None


def _get_program():
    global _PROGRAM
    if _PROGRAM is None:
        _PROGRAM = build_program()
    return _PROGRAM


def make_in_maps(q, k, v, W_O, n_cores=N_CORES):
    """Shard full inputs into per-core maps (core = batch*4 + head_group)."""
    import ml_dtypes
    mmdt = ml_dtypes.bfloat16
    q = np.ascontiguousarray(np.asarray(q, dtype=np.float32))
    k = np.ascontiguousarray(np.asarray(k, dtype=np.float32))
    v = np.ascontiguousarray(np.asarray(v, dtype=np.float32))
    W_O = np.ascontiguousarray(np.asarray(W_O, dtype=np.float32))
    B = q.shape[0]
    qT = np.ascontiguousarray(q.reshape(B, S, N_HEADS, D).transpose(0, 2, 3, 1))
    kT = np.ascontiguousarray(k.reshape(B, S, N_HEADS, D).transpose(0, 2, 3, 1))
    # v extended with a ones column (softmax denominator row) and pre-arranged
    # to the on-chip [partition, kv_tile, d+1] layout so the DMA is contiguous
    vh = v.reshape(B, S, N_HEADS, D).transpose(0, 2, 1, 3)  # [B, H, S, D]
    vext = np.concatenate(
        [vh, np.ones((B, N_HEADS, S, 1), dtype=np.float32)], axis=3
    ).reshape(B, N_HEADS, 16, P, D + 1).transpose(0, 1, 3, 2, 4)  # [B, Hh, P, 16, D+1]
    # mask[kv, q] = 1 iff kv <= q  (scores live transposed: partition=kv, free=q)
    tri = np.triu(np.ones((P, P), dtype=np.float32))
    tri2 = np.ascontiguousarray(np.concatenate([tri, tri], axis=1))
    in_maps = []
    for core in range(n_cores):
        b, g = core // 4, core % 4
        hs = slice(H * g, H * (g + 1))
        in_maps.append(
            {
                "qT": np.ascontiguousarray(qT[b, hs]).astype(mmdt),
                "kT": np.ascontiguousarray(kT[b, hs]).astype(mmdt),
                "vx": np.ascontiguousarray(vext[b, hs]).astype(mmdt),
                "wo": np.ascontiguousarray(W_O[hs].reshape(2 * P, D_MODEL)).astype(mmdt),
                "tri2": tri2.astype(mmdt),
            }
        )
    return in_maps


def kernel(residual, q, k, v, W_O, b_O, _trace=False, _trace_kwargs=None):
    global LAST_RESULTS
    residual = np.asarray(residual, dtype=np.float32)
    B = residual.shape[0]
    in_maps = make_in_maps(q, k, v, W_O)
    nc = _get_program()
    res = run_bass_kernel_spmd(
        nc, in_maps, list(range(N_CORES)), trace=_trace, **(_trace_kwargs or {})
    )
    LAST_RESULTS = res
    out = np.zeros((B, S, D_MODEL), dtype=np.float32)
    for core in range(N_CORES):
        out[core // 4] += res.results[core]["pout"].astype(np.float32)
    out += np.asarray(b_O, dtype=np.float32)
    return (residual, out.astype(np.float32))
